# revision 30
# baseline (speedup 1.0000x reference)
"""PointNet feature-propagation module on 8 Trainium2 cores.

Reference computation (per batch):
  dist, idx = 3-NN of xyz1 (n=4096) in xyz2 (m=1024)
  dist clamped to [0, 1e-10]  -> interpolation weights are exactly w=1/3
  interp = sum_k w * points2[idx_k]                    (n, 512)
  feat = [interp, points1] @ W1^T -> BN -> ReLU        (n, 256)
  feat = feat @ W2^T -> BN -> ReLU                     (n, 256)
  out = feat^T                                         (256, n)
BN statistics are over (batch, n) across ALL 16 batches -> AllReduce.

Strategy (data-parallel, 2 batches/core), v2 pipelined:
  - 3-NN via threshold mask: per query n, tau = midpoint of the 3rd/4th
    smallest distance (top-8 of negdist via nc.vector.max), then
    mask[m, n] selects the 3 nearest.  The gather+interp+first-half
    matmul collapses into y1a^T = Z @ mask with Z = points2@(0.5*w*W1a)^T.
    m-tiles 0..DVE_MTS-1 build the mask on DVE as {0,2} (exact, no bias);
    the rest build it on ACT as Sign {-1,+1} with a host-side colsum(Z)
    correction folded into the PSUM-evacuation bias.
  - tau is produced per GROUP of 4 n-tiles (h-chunk granularity) so
    phase E pipelines with pass 1; batch 1's pass 1 is interleaved with
    batch 0's phase E at emission level.
  - BN1 sums come free from the y1 evacuation (ACT accum_out); sumsq via
    an ACT Square pass.  BN2 stats via DVE bn_stats.  Each batch's
    partial stats AllReduce independently (the first hides under the
    other batch's compute).
  - Distances on the PE with fp32-grade precision via 3-term fp16
    coordinate splits (products exact in fp32); feature matmuls in fp16.
"""
import numpy as np

import concourse.bass as bass
import concourse.bacc as bacc
import concourse.tile as tile
import concourse.mybir as mybir
import concourse.bass_utils as bass_utils

F32 = mybir.dt.float32
F16 = mybir.dt.float16
AF = mybir.ActivationFunctionType
ALU = mybir.AluOpType
AX = mybir.AxisListType

N_CORES = 8
B_PER_CORE = 2
N = 4096          # query points per batch
M = 1024          # source points per batch
C1 = 256          # points1 channels
C2 = 512          # points2 channels
O = 256           # conv output channels
NT = N // 128     # 32 n-tiles
MT = M // 128     # 8 m-tiles
H = 512           # n-chunk (4 n-tiles)
NH = N // H       # 8 chunks
NG = 4            # n-tiles per group (= per h-chunk)
KROWS = 24        # K rows of the distance matmuls (21 data + 3 tau)
KD = 21           # rows without tau
EPS_BN = 1e-5
N_DVE_MTS = 3     # m-tiles 0..N_DVE_MTS-1 masked on DVE ({0,2}); rest ACT Sign
NSAMP = float(N)              # BN samples per batch (per-batch AllReduce)
NTOT = NSAMP * B_PER_CORE * N_CORES

ABLATE = set()   # dev-only: {"masks","accum","max","norm","evac","out"}
_PROGRAM_CACHE = {}


def _split3(x32):
    """3-term fp16 split: x ~ a+b+c with ~2^-33 relative error."""
    a = x32.astype(np.float16)
    r1 = x32 - a.astype(np.float32)
    b = r1.astype(np.float16)
    r2 = r1 - b.astype(np.float32)
    c = r2.astype(np.float16)
    return a, b, c


def _build_sides(x1, x2):
    """Build the K-row operands for the two distance matmuls.

    negdist'[n, m] = 2*x1[n]@x2[m] - |x2[m]|^2   (|x1|^2 dropped: constant
    per n, does not affect the per-n ranking over m).

    Product pairs per coordinate (u,v,w = x1 splits; a,b,c = x2 splits):
      (2u|a) (2v|a) (2u|b) (2v|b) (2w|a) (2u|c)
    Rows 18-20 carry -|x2|^2 as a 3-term split, rows 21-23 carry -tau
    (x1-side values filled on device).
    """
    n, m = x1.shape[0], x2.shape[0]
    s1 = np.zeros((KROWS, n), np.float16)
    s2 = np.zeros((KROWS, m), np.float16)
    for ci in range(3):
        u, v, w = _split3(x1[:, ci].astype(np.float32))
        a, b, c = _split3(x2[:, ci].astype(np.float32))
        r = 6 * ci
        s1[r + 0], s2[r + 0] = 2.0 * u, a
        s1[r + 1], s2[r + 1] = 2.0 * v, a
        s1[r + 2], s2[r + 2] = 2.0 * u, b
        s1[r + 3], s2[r + 3] = 2.0 * v, b
        s1[r + 4], s2[r + 4] = 2.0 * w, a
        s1[r + 5], s2[r + 5] = 2.0 * u, c
    x2f = x2.astype(np.float32)
    S = (x2f[:, 0] * x2f[:, 0] + x2f[:, 1] * x2f[:, 1]) + x2f[:, 2] * x2f[:, 2]
    sa, sb, sc = _split3(S)
    s1[18:21] = -1.0
    s2[18], s2[19], s2[20] = sa, sb, sc
    s2[21:24] = -1.0
    return s1, s2


def build_program(dbg=False, repeat=1, timing=False):
    key = ("nc", dbg, repeat, timing)
    if key in _PROGRAM_CACHE:
        return _PROGRAM_CACHE[key]
    nc = bacc.Bacc("TRN2", target_bir_lowering=False, debug=False,
                   num_devices=N_CORES)
    B = B_PER_CORE
    big = "Internal" if timing else "ExternalInput"
    x1s_d = nc.dram_tensor("x1s", [B, KROWS, N], F16, kind=big)
    x2s_d = nc.dram_tensor("x2s", [B, KROWS, M], F16, kind=big)
    p1T_d = nc.dram_tensor("p1T", [B, C1, N], F16, kind=big)
    p2T_d = nc.dram_tensor("p2T", [B, C2, M], F16, kind=big)
    zw_d = nc.dram_tensor("zw", [C2, O], F16, kind="ExternalInput")
    w1bT_d = nc.dram_tensor("w1bT", [C1, O], F16, kind="ExternalInput")
    w2T_d = nc.dram_tensor("w2T", [O, O], F16, kind="ExternalInput")
    gb1_d = nc.dram_tensor("gb1", [128, 4], F32, kind="ExternalInput")
    gb2_d = nc.dram_tensor("gb2", [128, 4], F32, kind="ExternalInput")
    csr_d = nc.dram_tensor("csr", [B, O], F16, kind="ExternalInput")
    ident_d = nc.dram_tensor("ident", [128, 128], F16, kind="ExternalInput")
    out_d = nc.dram_tensor("out", [B, O, N], F32,
                           kind="Internal" if timing else "ExternalOutput")
    if timing:
        tout_d = nc.dram_tensor("tout", [128, 2], F32, kind="ExternalOutput")

    from contextlib import ExitStack
    with tile.TileContext(nc) as tc:
        with ExitStack() as stack:
            ent = stack.enter_context
            consts = ent(tc.tile_pool(name="consts", bufs=1))
            inp = ent(tc.tile_pool(name="inp", bufs=3))
            p1pool = ent(tc.tile_pool(name="p1pool", bufs=2))
            p2pool = ent(tc.tile_pool(name="p2pool", bufs=2))
            zpool = ent(tc.tile_pool(name="zpool", bufs=2))
            ybig = ent(tc.tile_pool(name="ybig", bufs=1))
            y2big = ent(tc.tile_pool(name="y2big", bufs=1))
            masks = ent(tc.tile_pool(name="masks", bufs=6))
            small = ent(tc.tile_pool(name="small", bufs=3))
            stats = ent(tc.tile_pool(name="stats", bufs=2))
            trashp = ent(tc.tile_pool(name="trash", bufs=2))
            ostage = ent(tc.tile_pool(name="ostage", bufs=3))
            drampool = ent(tc.tile_pool(name="dram", bufs=2, space="DRAM"))
            psD1 = ent(tc.tile_pool(name="psD1", bufs=2, space="PSUM"))
            psScr = ent(tc.tile_pool(name="psScr", bufs=2, space="PSUM"))
            psY = ent(tc.tile_pool(name="psY", bufs=1, space="PSUM"))
            # ---- constants ----
            zw_sb = consts.tile([128, C2 // 128, O], F16)
            nc.sync.dma_start(zw_sb[:], zw_d.ap().rearrange(
                "(k p) o -> p k o", p=128))
            w1bT_sb = consts.tile([128, C1 // 128, O], F16)
            nc.sync.dma_start(w1bT_sb[:], w1bT_d.ap().rearrange(
                "(k p) o -> p k o", p=128))
            w2T_sb = consts.tile([128, O // 128, O], F16)
            nc.sync.dma_start(w2T_sb[:], w2T_d.ap().rearrange(
                "(k p) o -> p k o", p=128))
            gb1_sb = consts.tile([128, 4], F32)
            nc.sync.dma_start(gb1_sb[:], gb1_d.ap())
            gb2_sb = consts.tile([128, 4], F32)
            nc.sync.dma_start(gb2_sb[:], gb2_d.ap())
            ident_sb = consts.tile([128, 128], F16)
            nc.sync.dma_start(ident_sb[:], ident_d.ap())
            ones16 = consts.tile([1, H], F16)
            nc.gpsimd.memset(ones16[:], 1.0)
            pre_msks = []
            if "masks" in ABLATE:
                for _mi in range(4):
                    _mt = masks.tile([128, H], F16, tag="msk", name=f"mi{_mi}")
                    nc.gpsimd.memset(_mt[:], 1.0)
                    pre_msks.append(_mt)
            if timing:
                zt = consts.tile([128, 4096], F16)
                nc.gpsimd.memset(zt[:], 0.0)
                for t_d in (x1s_d, x2s_d, p1T_d, p2T_d):
                    flat = t_d.ap().rearrange("a b c -> (a b c)")
                    total = flat.shape[0]
                    csz = 128 * 4096
                    for off in range(0, total, csz):
                        ln = min(csz, total - off)
                        nc.sync.dma_start(
                            flat[off:off + ln].rearrange(
                                "(p f) -> p f", p=128),
                            zt[:, 0:ln // 128])

            for _rep in range(repeat):
                # ---- per-batch input DMAs (phase-ordered) ----
                x1s_l, x2s_l, p1T_l, z_l, cs_l = [], [], [], [], []
                p2T_l = []
                for b in range(B):
                    x1s = inp.tile([KROWS, N], F16, tag="x1s")
                    nc.sync.dma_start(x1s[0:KD, :], x1s_d.ap()[b][0:KD, :])
                    x2s = inp.tile([KROWS, M], F16, tag="x2s")
                    nc.sync.dma_start(x2s[:], x2s_d.ap()[b])
                    p2T = p2pool.tile([128, C2 // 128, M], F16, tag="p2T")
                    nc.sync.dma_start(p2T[:], p2T_d.ap()[b].rearrange(
                        "(k p) m -> p k m", p=128))
                    p2T_l.append(p2T)
                    cs_sb = small.tile([1, O], F16, tag="cs_sb")
                    nc.sync.dma_start(cs_sb[:], csr_d.ap()[b:b + 1, :])
                    x1s_l.append(x1s)
                    x2s_l.append(x2s)
                    cs_l.append(cs_sb)

                def emit_z(b):
                    p2T = p2T_l[b]
                    # ---- Z = points2 @ (0.5*w*W1a)^T  (m, o) fp16 ----
                    z_sb = zpool.tile([128, MT, O], F16, tag="z")
                    z_l.append(z_sb)
                    for mp in range(MT // 2):
                        z_ps = psD1.tile([128, 2 * O], F32, tag="d1",
                                         name=f"zps{b}_{mp}")
                        for half in range(2):
                            mt = 2 * mp + half
                            for kt in range(C2 // 128):
                                nc.tensor.matmul(
                                    z_ps[:, half * O:(half + 1) * O],
                                    p2T[:, kt, mt * 128:(mt + 1) * 128],
                                    zw_sb[:, kt, :],
                                    start=(kt == 0),
                                    stop=(kt == C2 // 128 - 1))
                        nc.scalar.copy(
                            z_sb[:, 2 * mp:2 * mp + 2, :],
                            z_ps[:].rearrange("p (a o) -> p a o", a=2))

                for b in range(B):
                    p1T = p1pool.tile([128, C1 // 128, N], F16, tag="p1T")
                    for hh in range(2):
                        sl = slice(hh * 2048, (hh + 1) * 2048)
                        nc.sync.dma_start(
                            p1T[:, :, sl],
                            p1T_d.ap()[b].rearrange(
                                "(k p) n -> p k n", p=128)[:, :, sl])
                    p1T_l.append(p1T)

                # ---------- pipeline helpers ----------
                def pass1_group(b, g):
                    """Distances + top-8 + tau for n-tiles [4g, 4g+4)."""
                    x1s, x2s = x1s_l[b], x2s_l[b]
                    strip = small.tile([128, NG * 8], F32, tag="strip")
                    if "max" in ABLATE:
                        nc.gpsimd.memset(strip[:], 0.5)
                    for j in range(NG):
                        nt = g * NG + j
                        d1 = psD1.tile([128, M], F32, tag="d1",
                                       name=f"d1_{b}_{nt}")
                        for half in range(2):
                            nc.tensor.matmul(
                                d1[:, half * 512:(half + 1) * 512],
                                x1s[0:KD, nt * 128:(nt + 1) * 128],
                                x2s[0:KD, half * 512:(half + 1) * 512],
                                start=True, stop=True)
                        if "max" not in ABLATE:
                            nc.vector.max(strip[:, j * 8:j * 8 + 8], d1[:])
    # tau = (v2 + v3) / 2 in [q, j] layout; 3-term fp16 split packed
                    # as [th | tl | t3] in a [128, 12] f16 tile, then one PE
                    # transpose -> [12, 128] and three plain-sliced DMAs.
                    sv = strip[:, :].rearrange("p (t e) -> p t e", e=8)
                    tau32 = small.tile([128, NG], F32, tag="tau32")
                    nc.vector.tensor_tensor(tau32[:], sv[:, :, 2],
                                            sv[:, :, 3], ALU.add)
                    nc.vector.tensor_scalar(tau32[:], tau32[:],
                                            0.5, None, ALU.mult)
                    pk = small.tile([128, 3 * NG], F16, tag="pk")
                    nc.vector.tensor_copy(pk[:, 0:NG], tau32[:])
                    r1 = small.tile([128, NG], F32, tag="r1")
                    nc.vector.tensor_tensor(r1[:], tau32[:], pk[:, 0:NG],
                                            ALU.subtract)
                    nc.vector.tensor_copy(pk[:, NG:2 * NG], r1[:])
                    r2 = small.tile([128, NG], F32, tag="r2")
                    nc.vector.tensor_tensor(r2[:], r1[:], pk[:, NG:2 * NG],
                                            ALU.subtract)
                    nc.vector.tensor_copy(pk[:, 2 * NG:3 * NG], r2[:])
                    tps = psD1.tile([3 * NG, 128], F16, tag="d1",
                                    name=f"tps_{b}_{g}")
                    nc.tensor.matmul(tps[:], pk[:], ident_sb[:],
                                     is_transpose=True)
                    tf = small.tile([3 * NG, 128], F16, tag="tf")
                    nc.vector.tensor_copy(tf[:], tps[:])
                    # row s*NG+j of tf = split s of tau for n-tile g*NG+j
                    for s in range(3):
                        nc.sync.dma_start(
                            x1s[KD + s:KD + s + 1,
                                g * H:(g + 1) * H].rearrange(
                                "a (j p) -> a j p", j=NG, p=128),
                            tf[s * NG:(s + 1) * NG, :])

                def phase_e_h(b, h):
                    """Mask + y1 accumulation for one h-chunk."""
                    x1s, x2s, p1T = x1s_l[b], x2s_l[b], p1T_l[b]
                    z_sb, cs_sb = z_l[b], cs_l[b]
                    y1_sb = y1_l[b]
                    hs = slice(h * H, (h + 1) * H)
                    py = psY.tile([128, 2 * H], F32, tag="py",
                                  name=f"py_{b}_{h}")
                    for mt in range(MT):
                        d2 = psScr.tile([128, H], F32, tag="scr",
                                        name=f"d2_{b}_{h}_{mt}")
                        nc.tensor.matmul(
                            d2[:], x2s[:, mt * 128:(mt + 1) * 128],
                            x1s[:, hs], start=True, stop=True)
                        msk = masks.tile([128, H], F16, tag="msk")
                        if "masks" in ABLATE:
                            msk = pre_msks[mt % 4]
                        elif mt < N_DVE_MTS:
                            nc.vector.tensor_scalar(
                                msk[:], d2[:], 0.0, 2.0,
                                ALU.is_gt, ALU.mult)
                        else:
                            nc.scalar.activation(msk[:], d2[:], AF.Sign)
                        for ot in range(2):
                            nc.tensor.matmul(
                                py[:, ot * H:(ot + 1) * H],
                                z_sb[:, mt, ot * 128:(ot + 1) * 128],
                                msk[:], start=(mt == 0), stop=False)
                    # colsum(Z) correction as a K=1 accumulation row
                    for ot in range(2):
                        nc.tensor.matmul(
                            py[:, ot * H:(ot + 1) * H],
                            cs_sb[0:1, ot * 128:(ot + 1) * 128],
                            ones16[0:1, :], start=False, stop=False)
                    for kt in range(C1 // 128):
                        for ot in range(2):
                            nc.tensor.matmul(
                                py[:, ot * H:(ot + 1) * H],
                                w1bT_sb[:, kt, ot * 128:(ot + 1) * 128],
                                p1T[:, kt, hs],
                                start=False, stop=(kt == C1 // 128 - 1))
                    # single plain evacuation (no bias needed)
                    pyv = py[:].rearrange("p (a n) -> p a n", a=2)
                    nc.scalar.copy(y1_sb[:, :, hs], pyv)
                    for ot in range(2):
                        nc.vector.bn_stats(
                            bn1_l[b][:, ot, h * 6:(h + 1) * 6],
                            y1_sb[:, ot, hs])

                # ---------- emission: pass1(b0), then E(b0) || pass1(b1) ----
                y1_l = [ybig.tile([128, 2, N], F16, tag=f"y1_{b}", name=f"y1_{b}")
                        for b in range(B)]
                bn1_l = [stats.tile([128, 2, NH * 6], F32, tag=f"bn1_{b}",
                                    name=f"bn1_{b}") for b in range(B)]
                bn2_l = [stats.tile([128, 2, NH * 6], F32, tag=f"bn2_{b}",
                                    name=f"bn2_{b}") for b in range(B)]

                pass1_group(0, 0)
                pass1_group(0, 1)
                emit_z(0)
                for g in range(2, NH):
                    pass1_group(0, g)
                emit_z(1)
                ar_out = [None, None]
                for s in range(NH):
                    pass1_group(1, s)
                    phase_e_h(0, s)
                    if s == NH - 1:
                        # batch 0 stats AllReduce (hides under b1 phase E)
                        ar_out[0] = _bn_ar(nc, small, drampool,
                                           bn1_l[0], "bn1a")
                    if s >= 1:
                        phase_e_h(1, s - 1)
                phase_e_h(1, NH - 1)
                ar_out[1] = _bn_ar(nc, small, drampool, bn1_l[1], "bn1b")
                s1_sb, t1_sb = _stats_finish(nc, small, ar_out, gb1_sb, "bn1")

                # ---- normalize (in place) + layer 2 + BN2 stats ----
                y2_l = []
                for b in range(B):
                    y1_sb = y1_l[b]
                    y2_sb = y2big.tile([128, 2, N], F16, tag=f"y2_{b}",
                                       name=f"y2_{b}")
                    y2_l.append(y2_sb)
                    for h in range(NH):
                        hs = slice(h * H, (h + 1) * H)
                        for kt in range(2):
                            if "norm" in ABLATE:
                                continue
                            nc.vector.tensor_scalar(
                                y1_sb[:, kt, hs], y1_sb[:, kt, hs],
                                s1_sb[:, kt:kt + 1], t1_sb[:, kt:kt + 1],
                                ALU.mult, ALU.add)
                            nc.vector.tensor_scalar(
                                y1_sb[:, kt, hs], y1_sb[:, kt, hs],
                                0.0, None, ALU.max)
                        p2y = psY.tile([128, 2 * H], F32, tag="py",
                                       name=f"p2y_{b}_{h}")
                        for ot2 in range(2):
                            for kt in range(2):
                                nc.tensor.matmul(
                                    p2y[:, ot2 * H:(ot2 + 1) * H],
                                    w2T_sb[:, kt, ot2 * 128:(ot2 + 1) * 128],
                                    y1_sb[:, kt, hs],
                                    start=(kt == 0), stop=(kt == 1))
                        p2yv = p2y[:].rearrange("p (a n) -> p a n", a=2)
                        nc.scalar.copy(y2_sb[:, :, hs], p2yv)
                        for ot2 in range(2):
                            nc.vector.bn_stats(
                                bn2_l[b][:, ot2, h * 6:(h + 1) * 6],
                                y2_sb[:, ot2, hs])
                    ar_out[b] = _bn_ar(nc, small, drampool, bn2_l[b],
                                       f"bn2{b}")
                s2_sb, t2_sb = _stats_finish(nc, small, ar_out, gb2_sb, "bn2")

                # ---- final normalize + output ----
                for b in range(B):
                    y2_sb = y2_l[b]
                    for ot2 in range(2):
                        for oh in range(4):
                            osl = slice(oh * 1024, (oh + 1) * 1024)
                            ot_out = ostage.tile([128, 1024], F32, tag="ost")
                            nc.scalar.activation(ot_out[:], y2_sb[:, ot2, osl],
                                                 AF.Relu,
                                                 scale=s2_sb[:, ot2:ot2 + 1],
                                                 bias=t2_sb[:, ot2:ot2 + 1])
                            nc.gpsimd.dma_start(
                                out_d.ap()[b][ot2 * 128:(ot2 + 1) * 128, osl],
                                ot_out[:])
                if timing and _rep == repeat - 1:
                    nc.sync.dma_start(tout_d.ap(), s2_sb[:])
    nc.compile()
    _PROGRAM_CACHE[key] = nc
    return nc


def _bn_ar(nc, small, drampool, strip, name):
    """bn_stats aggregation -> sums -> this batch's AllReduce."""
    arin = small.tile([128, 4], F32, tag=f"ar_in_{name}")
    for ot in range(2):
        agg = small.tile([128, 2], F32, tag=f"{name}_agg")
        nc.vector.bn_aggr(agg[:], strip[:, ot, :])
        nc.vector.tensor_scalar(arin[:, 2 * ot:2 * ot + 1], agg[:, 0:1],
                                NSAMP, None, ALU.mult)
        m2 = small.tile([128, 1], F32, tag=f"{name}_m2")
        nc.vector.tensor_tensor(m2[:], agg[:, 0:1], agg[:, 0:1], ALU.mult)
        sqs = small.tile([128, 1], F32, tag=f"{name}_sq")
        nc.vector.tensor_tensor(sqs[:], agg[:, 1:2], m2[:], ALU.add)
        nc.vector.tensor_scalar(arin[:, 2 * ot + 1:2 * ot + 2], sqs[:],
                                NSAMP, None, ALU.mult)
    din = drampool.tile([128, 4], F32, tag=f"din_{name}")
    dout = drampool.tile([128, 4], F32, tag=f"dout_{name}")
    nc.gpsimd.dma_start(din[:], arin[:])
    nc.gpsimd.collective_compute(
        "AllReduce", ALU.add, replica_groups=[list(range(N_CORES))],
        ins=[din.opt()], outs=[dout.opt()])
    return dout


def _stats_finish(nc, small, ar_outs, gb_sb, name):
    """Combine the two per-batch AllReduce results into scale/shift."""
    ags = []
    for i, dout in enumerate(ar_outs):
        ag = small.tile([128, 4], F32, tag=f"{name}_ag{i}")
        nc.gpsimd.dma_start(ag[:], dout[:])
        ags.append(ag)
    tot = small.tile([128, 4], F32, tag=f"{name}_tot")
    nc.vector.tensor_tensor(tot[:], ags[0][:], ags[1][:], ALU.add)

    s_sb = small.tile([128, 2], F32, tag=f"{name}_s")
    t_sb = small.tile([128, 2], F32, tag=f"{name}_t")
    for ot in range(2):
        mean = small.tile([128, 1], F32, tag=f"{name}_mean")
        nc.vector.tensor_scalar(mean[:], tot[:, 2 * ot:2 * ot + 1],
                                1.0 / NTOT, None, ALU.mult)
        ey2 = small.tile([128, 1], F32, tag=f"{name}_ey2")
        nc.vector.tensor_scalar(ey2[:], tot[:, 2 * ot + 1:2 * ot + 2],
                                1.0 / NTOT, None, ALU.mult)
        m2 = small.tile([128, 1], F32, tag=f"{name}_gm2")
        nc.vector.tensor_tensor(m2[:], mean[:], mean[:], ALU.mult)
        x = small.tile([128, 1], F32, tag=f"{name}_x")
        nc.vector.scalar_tensor_tensor(x[:], ey2[:], EPS_BN, m2[:],
                                       ALU.add, ALU.subtract)
        # sqrt + 2 Newton steps (ACT Sqrt alone can be inaccurate)
        sd = small.tile([128, 1], F32, tag=f"{name}_sd")
        nc.scalar.activation(sd[:], x[:], AF.Sqrt)
        for it in range(2):
            rc = small.tile([128, 1], F32, tag=f"{name}_rc{it}")
            nc.vector.reciprocal(rc[:], sd[:])
            q = small.tile([128, 1], F32, tag=f"{name}_q{it}")
            nc.vector.tensor_tensor(q[:], x[:], rc[:], ALU.mult)
            u = small.tile([128, 1], F32, tag=f"{name}_u{it}")
            nc.vector.tensor_tensor(u[:], sd[:], q[:], ALU.add)
            sd = small.tile([128, 1], F32, tag=f"{name}_sd{it}")
            nc.vector.tensor_scalar(sd[:], u[:], 0.5, None, ALU.mult)
        inv = small.tile([128, 1], F32, tag=f"{name}_inv")
        nc.vector.reciprocal(inv[:], sd[:])
        nc.vector.tensor_tensor(s_sb[:, ot:ot + 1], inv[:],
                                gb_sb[:, 2 * ot:2 * ot + 1], ALU.mult)
        ms = small.tile([128, 1], F32, tag=f"{name}_ms")
        nc.vector.tensor_tensor(ms[:], mean[:], s_sb[:, ot:ot + 1], ALU.mult)
        nc.vector.tensor_tensor(t_sb[:, ot:ot + 1],
                                gb_sb[:, 2 * ot + 1:2 * ot + 2], ms[:],
                                ALU.subtract)
    return s_sb, t_sb


def _prep_core(xyz1, xyz2, points1, points2):
    """Host-side prep of one core's 2 batches."""
    B = xyz1.shape[0]
    x1s = np.zeros((B, KROWS, N), np.float16)
    x2s = np.zeros((B, KROWS, M), np.float16)
    for b in range(B):
        s1, s2 = _build_sides(xyz1[b], xyz2[b])
        x1s[b], x2s[b] = s1, s2
    p1T = np.ascontiguousarray(points1.transpose(0, 2, 1)).astype(np.float16)
    p2T = np.ascontiguousarray(points2.transpose(0, 2, 1)).astype(np.float16)
    return x1s, x2s, p1T, p2T


def _csr(p2, zw):
    """colsum(Z) over the ACT-Sign m-tiles only (m >= 128*N_DVE_MTS)."""
    m0 = 128 * N_DVE_MTS
    out = np.zeros((p2.shape[0], O), np.float16)
    for b in range(p2.shape[0]):
        cs = p2[b, m0:].astype(np.float16).astype(np.float32).sum(0) @ \
            zw.astype(np.float32)
        out[b] = cs.astype(np.float16)
    return out


def kernel(xyz1, xyz2, points1, points2, W1, b1, g1, beta1, W2, b2, g2,
           beta2):
    xyz1, xyz2 = np.asarray(xyz1), np.asarray(xyz2)
    points1, points2 = np.asarray(points1), np.asarray(points2)
    W1, W2 = np.asarray(W1, np.float32), np.asarray(W2, np.float32)
    g1, beta1 = np.asarray(g1, np.float32), np.asarray(beta1, np.float32)
    g2, beta2 = np.asarray(g2, np.float32), np.asarray(beta2, np.float32)
    # interpolation weight exactly as the reference computes it
    dist = np.float32(1e-10)
    inv = np.float32(1.0) / dist
    ssum = (inv + inv) + inv
    w = inv / ssum  # fp32(1/3)-ish, bit-exact vs reference

    zw = (0.5 * w * W1[:, :C2].astype(np.float32)).T.astype(np.float16)
    w1bT = np.ascontiguousarray(W1[:, C2:].T).astype(np.float16)
    w2T = np.ascontiguousarray(W2.T).astype(np.float16)
    # conv biases b1/b2 are no-ops through BN (mean subtracts them exactly)
    gb1 = np.stack([g1[0:128], beta1[0:128], g1[128:256], beta1[128:256]],
                   1).astype(np.float32)
    gb2 = np.stack([g2[0:128], beta2[0:128], g2[128:256], beta2[128:256]],
                   1).astype(np.float32)
    ident = np.eye(128, dtype=np.float16)

    nc = build_program()
    in_maps = []
    for c in range(N_CORES):
        bs = slice(c * B_PER_CORE, (c + 1) * B_PER_CORE)
        x1s, x2s, p1T, p2T = _prep_core(
            np.asarray(xyz1[bs]), np.asarray(xyz2[bs]),
            np.asarray(points1[bs]), np.asarray(points2[bs]))
        csr = _csr(np.asarray(points2[bs]), zw)
        in_maps.append(dict(x1s=x1s, x2s=x2s, p1T=p1T, p2T=p2T, zw=zw,
                            w1bT=w1bT, w2T=w2T, gb1=gb1, gb2=gb2, csr=csr,
                            ident=ident))
    res = bass_utils.run_bass_kernel_spmd(
        nc, in_maps, core_ids=list(range(N_CORES)), trace=False)
    out = np.concatenate([res.results[c]["out"] for c in range(N_CORES)],
                         axis=0)
    return out.astype(np.float32)


# revision 33
# speedup vs baseline: 1.4113x; 1.4113x over previous
"""PointNet feature-propagation module on 8 Trainium2 cores.

Reference computation (per batch):
  dist, idx = 3-NN of xyz1 (n=4096) in xyz2 (m=1024)
  dist clamped to [0, 1e-10]  -> interpolation weights are exactly w=1/3
  interp = sum_k w * points2[idx_k]                    (n, 512)
  feat = [interp, points1] @ W1^T -> BN -> ReLU        (n, 256)
  feat = feat @ W2^T -> BN -> ReLU                     (n, 256)
  out = feat^T                                         (256, n)
BN statistics are over (batch, n) across ALL 16 batches -> AllReduce.

Strategy (data-parallel, 2 batches/core), v2 pipelined:
  - 3-NN via threshold mask: per query n, tau = midpoint of the 3rd/4th
    smallest distance (top-8 of negdist via nc.vector.max), then
    mask[m, n] selects the 3 nearest.  The gather+interp+first-half
    matmul collapses into y1a^T = Z @ mask with Z = points2@(0.5*w*W1a)^T.
    m-tiles 0..DVE_MTS-1 build the mask on DVE as {0,2} (exact, no bias);
    the rest build it on ACT as Sign {-1,+1} with a host-side colsum(Z)
    correction folded into the PSUM-evacuation bias.
  - tau is produced per GROUP of 4 n-tiles (h-chunk granularity) so
    phase E pipelines with pass 1; batch 1's pass 1 is interleaved with
    batch 0's phase E at emission level.
  - BN1 sums come free from the y1 evacuation (ACT accum_out); sumsq via
    an ACT Square pass.  BN2 stats via DVE bn_stats.  Each batch's
    partial stats AllReduce independently (the first hides under the
    other batch's compute).
  - Distances on the PE with fp32-grade precision via 3-term fp16
    coordinate splits (products exact in fp32); feature matmuls in fp16.
"""
import numpy as np

import concourse.bass as bass
import concourse.bacc as bacc
import concourse.tile as tile
import concourse.mybir as mybir
import concourse.bass_utils as bass_utils

F32 = mybir.dt.float32
F16 = mybir.dt.float16
AF = mybir.ActivationFunctionType
ALU = mybir.AluOpType
AX = mybir.AxisListType

N_CORES = 8
B_PER_CORE = 2
N = 4096          # query points per batch
M = 1024          # source points per batch
C1 = 256          # points1 channels
C2 = 512          # points2 channels
O = 256           # conv output channels
NT = N // 128     # 32 n-tiles
MT = M // 128     # 8 m-tiles
H = 512           # n-chunk (4 n-tiles)
NH = N // H       # 8 chunks
NG = 4            # n-tiles per group (= per h-chunk)
KROWS = 24        # K rows of the distance matmuls (21 data + 3 tau)
KD = 21           # rows without tau
EPS_BN = 1e-5
N_DVE_MTS = 2     # m-tiles 0..N_DVE_MTS-1 masked on DVE ({0,2}); rest ACT Sign
NSAMP = float(N)              # BN samples per batch (per-batch AllReduce)
NTOT = NSAMP * B_PER_CORE * N_CORES

ABLATE = set()   # dev-only: {"masks","accum","max","norm","evac","out"}
_PROGRAM_CACHE = {}


def _split3(x32):
    """3-term fp16 split: x ~ a+b+c with ~2^-33 relative error."""
    a = x32.astype(np.float16)
    r1 = x32 - a.astype(np.float32)
    b = r1.astype(np.float16)
    r2 = r1 - b.astype(np.float32)
    c = r2.astype(np.float16)
    return a, b, c


def _build_sides(x1, x2):
    """Build the K-row operands for the two distance matmuls.

    negdist'[n, m] = 2*x1[n]@x2[m] - |x2[m]|^2   (|x1|^2 dropped: constant
    per n, does not affect the per-n ranking over m).

    Product pairs per coordinate (u,v,w = x1 splits; a,b,c = x2 splits):
      (2u|a) (2v|a) (2u|b) (2v|b) (2w|a) (2u|c)
    Rows 18-20 carry -|x2|^2 as a 3-term split, rows 21-23 carry -tau
    (x1-side values filled on device).
    """
    n, m = x1.shape[0], x2.shape[0]
    s1 = np.zeros((KROWS, n), np.float16)
    s2 = np.zeros((KROWS, m), np.float16)
    for ci in range(3):
        u, v, w = _split3(x1[:, ci].astype(np.float32))
        a, b, c = _split3(x2[:, ci].astype(np.float32))
        r = 6 * ci
        s1[r + 0], s2[r + 0] = 2.0 * u, a
        s1[r + 1], s2[r + 1] = 2.0 * v, a
        s1[r + 2], s2[r + 2] = 2.0 * u, b
        s1[r + 3], s2[r + 3] = 2.0 * v, b
        s1[r + 4], s2[r + 4] = 2.0 * w, a
        s1[r + 5], s2[r + 5] = 2.0 * u, c
    x2f = x2.astype(np.float32)
    S = (x2f[:, 0] * x2f[:, 0] + x2f[:, 1] * x2f[:, 1]) + x2f[:, 2] * x2f[:, 2]
    sa, sb, sc = _split3(S)
    s1[18:21] = -1.0
    s2[18], s2[19], s2[20] = sa, sb, sc
    s2[21:24] = -1.0
    return s1, s2


def build_program(dbg=False, repeat=1, timing=False):
    key = ("nc", dbg, repeat, timing)
    if key in _PROGRAM_CACHE:
        return _PROGRAM_CACHE[key]
    nc = bacc.Bacc("TRN2", target_bir_lowering=False, debug=False,
                   num_devices=N_CORES)
    B = B_PER_CORE
    big = "Internal" if timing else "ExternalInput"
    x1s_d = nc.dram_tensor("x1s", [B, KROWS, N], F16, kind=big)
    x2s_d = nc.dram_tensor("x2s", [B, KROWS, M], F16, kind=big)
    p1T_d = nc.dram_tensor("p1T", [B, C1, N], F16, kind=big)
    p2T_d = nc.dram_tensor("p2T", [B, C2, M], F16, kind=big)
    zw_d = nc.dram_tensor("zw", [C2, O], F16, kind="ExternalInput")
    w1bT_d = nc.dram_tensor("w1bT", [C1, O], F16, kind="ExternalInput")
    w2T_d = nc.dram_tensor("w2T", [O, O], F16, kind="ExternalInput")
    gb1_d = nc.dram_tensor("gb1", [128, 4], F32, kind="ExternalInput")
    gb2_d = nc.dram_tensor("gb2", [128, 4], F32, kind="ExternalInput")
    csr_d = nc.dram_tensor("csr", [B, O], F16, kind="ExternalInput")
    ident_d = nc.dram_tensor("ident", [128, 128], F16, kind="ExternalInput")
    out_d = nc.dram_tensor("out", [B, O, N], F32,
                           kind="Internal" if timing else "ExternalOutput")
    if timing:
        tout_d = nc.dram_tensor("tout", [128, 2], F32, kind="ExternalOutput")

    from contextlib import ExitStack
    with tile.TileContext(nc) as tc:
        with ExitStack() as stack:
            ent = stack.enter_context
            consts = ent(tc.tile_pool(name="consts", bufs=1))
            inp = ent(tc.tile_pool(name="inp", bufs=3))
            p1pool = ent(tc.tile_pool(name="p1pool", bufs=2))
            p2pool = ent(tc.tile_pool(name="p2pool", bufs=3))
            zpool = ent(tc.tile_pool(name="zpool", bufs=3))
            ybig = ent(tc.tile_pool(name="ybig", bufs=1))
            y2big = ent(tc.tile_pool(name="y2big", bufs=1))
            masks = ent(tc.tile_pool(name="masks", bufs=6))
            small = ent(tc.tile_pool(name="small", bufs=3))
            stats = ent(tc.tile_pool(name="stats", bufs=2))
            trashp = ent(tc.tile_pool(name="trash", bufs=2))
            ostage = ent(tc.tile_pool(name="ostage", bufs=3))
            drampool = ent(tc.tile_pool(name="dram", bufs=2, space="DRAM"))
            psD1 = ent(tc.tile_pool(name="psD1", bufs=2, space="PSUM"))
            psScr = ent(tc.tile_pool(name="psScr", bufs=2, space="PSUM"))
            psY = ent(tc.tile_pool(name="psY", bufs=1, space="PSUM"))
            # ---- constants ----
            zw_sb = consts.tile([128, C2 // 128, O], F16)
            nc.sync.dma_start(zw_sb[:], zw_d.ap().rearrange(
                "(k p) o -> p k o", p=128))
            w1bT_sb = consts.tile([128, C1 // 128, O], F16)
            nc.sync.dma_start(w1bT_sb[:], w1bT_d.ap().rearrange(
                "(k p) o -> p k o", p=128))
            w2T_sb = consts.tile([128, O // 128, O], F16)
            nc.sync.dma_start(w2T_sb[:], w2T_d.ap().rearrange(
                "(k p) o -> p k o", p=128))
            gb1_sb = consts.tile([128, 4], F32)
            nc.sync.dma_start(gb1_sb[:], gb1_d.ap())
            gb2_sb = consts.tile([128, 4], F32)
            nc.sync.dma_start(gb2_sb[:], gb2_d.ap())
            ident_sb = consts.tile([128, 128], F16)
            nc.sync.dma_start(ident_sb[:], ident_d.ap())
            ones16 = consts.tile([1, H], F16)
            nc.gpsimd.memset(ones16[:], 1.0)
            pre_msks = []
            if "masks" in ABLATE:
                for _mi in range(4):
                    _mt = masks.tile([128, H], F16, tag="msk", name=f"mi{_mi}")
                    nc.gpsimd.memset(_mt[:], 1.0)
                    pre_msks.append(_mt)
            if timing:
                zt = consts.tile([128, 4096], F16)
                nc.gpsimd.memset(zt[:], 0.0)
                for t_d in (x1s_d, x2s_d, p1T_d, p2T_d):
                    flat = t_d.ap().rearrange("a b c -> (a b c)")
                    total = flat.shape[0]
                    csz = 128 * 4096
                    for off in range(0, total, csz):
                        ln = min(csz, total - off)
                        nc.sync.dma_start(
                            flat[off:off + ln].rearrange(
                                "(p f) -> p f", p=128),
                            zt[:, 0:ln // 128])

            for _rep in range(repeat):
                # ---- per-batch input DMAs (phase-ordered) ----
                x1s_l, x2s_l, p1T_l, z_l, cs_l = [], [], [], [], []
                p2T_l = []
                for b in range(B):
                    x1s = inp.tile([KROWS, N], F16, tag="x1s")
                    nc.sync.dma_start(x1s[0:KD, :], x1s_d.ap()[b][0:KD, :])
                    x2s = inp.tile([KROWS, M], F16, tag="x2s")
                    nc.sync.dma_start(x2s[:], x2s_d.ap()[b])
                    p2T = p2pool.tile([128, C2 // 128, M], F16, tag="p2T")
                    nc.sync.dma_start(p2T[:], p2T_d.ap()[b].rearrange(
                        "(k p) m -> p k m", p=128))
                    p2T_l.append(p2T)
                    cs_sb = small.tile([1, O], F16, tag="cs_sb")
                    nc.sync.dma_start(cs_sb[:], csr_d.ap()[b:b + 1, :])
                    x1s_l.append(x1s)
                    x2s_l.append(x2s)
                    cs_l.append(cs_sb)

                def emit_z(b):
                    p2T = p2T_l[b]
                    # ---- Z = points2 @ (0.5*w*W1a)^T  (m, o) fp16 ----
                    z_sb = zpool.tile([128, MT, O], F16, tag="z")
                    z_l.append(z_sb)
                    for mp in range(MT // 2):
                        z_ps = psD1.tile([128, 2 * O], F32, tag="d1",
                                         name=f"zps{b}_{mp}")
                        for half in range(2):
                            mt = 2 * mp + half
                            for kt in range(C2 // 128):
                                nc.tensor.matmul(
                                    z_ps[:, half * O:(half + 1) * O],
                                    p2T[:, kt, mt * 128:(mt + 1) * 128],
                                    zw_sb[:, kt, :],
                                    start=(kt == 0),
                                    stop=(kt == C2 // 128 - 1))
                        nc.scalar.copy(
                            z_sb[:, 2 * mp:2 * mp + 2, :],
                            z_ps[:].rearrange("p (a o) -> p a o", a=2))

                for b in range(B):
                    p1T = p1pool.tile([128, C1 // 128, N], F16, tag="p1T")
                    for hh in range(2):
                        sl = slice(hh * 2048, (hh + 1) * 2048)
                        nc.sync.dma_start(
                            p1T[:, :, sl],
                            p1T_d.ap()[b].rearrange(
                                "(k p) n -> p k n", p=128)[:, :, sl])
                    p1T_l.append(p1T)

                # ---------- pipeline helpers ----------
                def pass1_group(b, g):
                    """Distances + top-8 + tau for n-tiles [4g, 4g+4)."""
                    x1s, x2s = x1s_l[b], x2s_l[b]
                    strip = small.tile([128, NG * 8], F32, tag="strip")
                    if "max" in ABLATE:
                        nc.gpsimd.memset(strip[:], 0.5)
                    for j in range(NG):
                        nt = g * NG + j
                        d1 = psD1.tile([128, M], F32, tag="d1",
                                       name=f"d1_{b}_{nt}")
                        for half in range(2):
                            nc.tensor.matmul(
                                d1[:, half * 512:(half + 1) * 512],
                                x1s[0:KD, nt * 128:(nt + 1) * 128],
                                x2s[0:KD, half * 512:(half + 1) * 512],
                                start=True, stop=True)
                        if "max" not in ABLATE:
                            nc.vector.max(strip[:, j * 8:j * 8 + 8], d1[:])
    # tau = (v2 + v3) / 2 in [q, j] layout; 3-term fp16 split packed
                    # as [th | tl | t3] in a [128, 12] f16 tile, then one PE
                    # transpose -> [12, 128] and three plain-sliced DMAs.
                    sv = strip[:, :].rearrange("p (t e) -> p t e", e=8)
                    tau32 = small.tile([128, NG], F32, tag="tau32")
                    nc.vector.tensor_tensor(tau32[:], sv[:, :, 2],
                                            sv[:, :, 3], ALU.add)
                    nc.vector.tensor_scalar(tau32[:], tau32[:],
                                            0.5, None, ALU.mult)
                    pk = small.tile([128, 3 * NG], F16, tag="pk")
                    nc.vector.tensor_copy(pk[:, 0:NG], tau32[:])
                    r1 = small.tile([128, NG], F32, tag="r1")
                    nc.vector.tensor_tensor(r1[:], tau32[:], pk[:, 0:NG],
                                            ALU.subtract)
                    nc.vector.tensor_copy(pk[:, NG:2 * NG], r1[:])
                    r2 = small.tile([128, NG], F32, tag="r2")
                    nc.vector.tensor_tensor(r2[:], r1[:], pk[:, NG:2 * NG],
                                            ALU.subtract)
                    nc.vector.tensor_copy(pk[:, 2 * NG:3 * NG], r2[:])
                    tps = psD1.tile([3 * NG, 128], F16, tag="d1",
                                    name=f"tps_{b}_{g}")
                    nc.tensor.matmul(tps[:], pk[:], ident_sb[:],
                                     is_transpose=True)
                    tf = small.tile([3 * NG, 128], F16, tag="tf")
                    nc.vector.tensor_copy(tf[:], tps[:])
                    # row s*NG+j of tf = split s of tau for n-tile g*NG+j
                    for s in range(3):
                        nc.sync.dma_start(
                            x1s[KD + s:KD + s + 1,
                                g * H:(g + 1) * H].rearrange(
                                "a (j p) -> a j p", j=NG, p=128),
                            tf[s * NG:(s + 1) * NG, :])

                def phase_e_h(b, h):
                    """Mask + y1 accumulation for one h-chunk."""
                    x1s, x2s, p1T = x1s_l[b], x2s_l[b], p1T_l[b]
                    z_sb, cs_sb = z_l[b], cs_l[b]
                    y1_sb = y1_l[b]
                    hs = slice(h * H, (h + 1) * H)
                    py = psY.tile([128, 2 * H], F32, tag="py",
                                  name=f"py_{b}_{h}")
                    for mt in range(MT):
                        d2 = psScr.tile([128, H], F32, tag="scr",
                                        name=f"d2_{b}_{h}_{mt}")
                        nc.tensor.matmul(
                            d2[:], x2s[:, mt * 128:(mt + 1) * 128],
                            x1s[:, hs], start=True, stop=True)
                        msk = masks.tile([128, H], F16, tag="msk")
                        if "masks" in ABLATE:
                            msk = pre_msks[mt % 4]
                        elif mt < N_DVE_MTS:
                            nc.vector.tensor_scalar(
                                msk[:], d2[:], 0.0, 2.0,
                                ALU.is_gt, ALU.mult)
                        else:
                            nc.scalar.activation(msk[:], d2[:], AF.Sign)
                        for ot in range(2):
                            nc.tensor.matmul(
                                py[:, ot * H:(ot + 1) * H],
                                z_sb[:, mt, ot * 128:(ot + 1) * 128],
                                msk[:], start=(mt == 0), stop=False)
                    # colsum(Z) correction as a K=1 accumulation row
                    for ot in range(2):
                        nc.tensor.matmul(
                            py[:, ot * H:(ot + 1) * H],
                            cs_sb[0:1, ot * 128:(ot + 1) * 128],
                            ones16[0:1, :], start=False, stop=False)
                    for kt in range(C1 // 128):
                        for ot in range(2):
                            nc.tensor.matmul(
                                py[:, ot * H:(ot + 1) * H],
                                w1bT_sb[:, kt, ot * 128:(ot + 1) * 128],
                                p1T[:, kt, hs],
                                start=False, stop=(kt == C1 // 128 - 1))
                    # single plain evacuation (no bias needed)
                    pyv = py[:].rearrange("p (a n) -> p a n", a=2)
                    nc.scalar.copy(y1_sb[:, :, hs], pyv)
                    for ot in range(2):
                        nc.vector.bn_stats(
                            bn1_l[b][:, ot, h * 6:(h + 1) * 6],
                            y1_sb[:, ot, hs])

                # ---------- emission: pass1(b0), then E(b0) || pass1(b1) ----
                y1_l = [ybig.tile([128, 2, N], F16, tag=f"y1_{b}", name=f"y1_{b}")
                        for b in range(B)]
                bn1_l = [stats.tile([128, 2, NH * 6], F32, tag=f"bn1_{b}",
                                    name=f"bn1_{b}") for b in range(B)]
                bn2_l = [stats.tile([128, 2, NH * 6], F32, tag=f"bn2_{b}",
                                    name=f"bn2_{b}") for b in range(B)]

                pass1_group(0, 0)
                pass1_group(0, 1)
                emit_z(0)
                for g in range(2, NH):
                    pass1_group(0, g)
                emit_z(1)
                for s in range(NH):
                    pass1_group(1, s)
                    phase_e_h(0, s)
                # batch 0 stats AllReduce (hides under batch 1 phase E)
                ar_out = [None, None]
                ar_out[0] = _bn_ar(nc, small, drampool, bn1_l[0], "bn1a")
                for s in range(NH):
                    phase_e_h(1, s)
                ar_out[1] = _bn_ar(nc, small, drampool, bn1_l[1], "bn1b")
                s1_sb, t1_sb = _stats_finish(nc, small, ar_out, gb1_sb, "bn1")

                # ---- normalize (in place) + layer 2 + BN2 stats ----
                y2_l = []
                for b in range(B):
                    y1_sb = y1_l[b]
                    y2_sb = y2big.tile([128, 2, N], F16, tag=f"y2_{b}",
                                       name=f"y2_{b}")
                    y2_l.append(y2_sb)
                    for h in range(NH):
                        hs = slice(h * H, (h + 1) * H)
                        for kt in range(2):
                            if "norm" in ABLATE:
                                continue
                            nc.vector.tensor_scalar(
                                y1_sb[:, kt, hs], y1_sb[:, kt, hs],
                                s1_sb[:, kt:kt + 1], t1_sb[:, kt:kt + 1],
                                ALU.mult, ALU.add)
                            nc.vector.tensor_scalar(
                                y1_sb[:, kt, hs], y1_sb[:, kt, hs],
                                0.0, None, ALU.max)
                        p2y = psY.tile([128, 2 * H], F32, tag="py",
                                       name=f"p2y_{b}_{h}")
                        for ot2 in range(2):
                            for kt in range(2):
                                nc.tensor.matmul(
                                    p2y[:, ot2 * H:(ot2 + 1) * H],
                                    w2T_sb[:, kt, ot2 * 128:(ot2 + 1) * 128],
                                    y1_sb[:, kt, hs],
                                    start=(kt == 0), stop=(kt == 1))
                        p2yv = p2y[:].rearrange("p (a n) -> p a n", a=2)
                        nc.scalar.copy(y2_sb[:, :, hs], p2yv)
                        for ot2 in range(2):
                            nc.vector.bn_stats(
                                bn2_l[b][:, ot2, h * 6:(h + 1) * 6],
                                y2_sb[:, ot2, hs])
                    ar_out[b] = _bn_ar(nc, small, drampool, bn2_l[b],
                                       f"bn2{b}")
                s2_sb, t2_sb = _stats_finish(nc, small, ar_out, gb2_sb, "bn2")

                # ---- final normalize + output ----
                for b in range(B):
                    y2_sb = y2_l[b]
                    for ot2 in range(2):
                        for oh in range(4):
                            osl = slice(oh * 1024, (oh + 1) * 1024)
                            ot_out = ostage.tile([128, 1024], F32, tag="ost")
                            nc.scalar.activation(ot_out[:], y2_sb[:, ot2, osl],
                                                 AF.Relu,
                                                 scale=s2_sb[:, ot2:ot2 + 1],
                                                 bias=t2_sb[:, ot2:ot2 + 1])
                            nc.gpsimd.dma_start(
                                out_d.ap()[b][ot2 * 128:(ot2 + 1) * 128, osl],
                                ot_out[:])
                if timing and _rep == repeat - 1:
                    nc.sync.dma_start(tout_d.ap(), s2_sb[:])
    nc.compile()
    _PROGRAM_CACHE[key] = nc
    return nc


def _bn_ar(nc, small, drampool, strip, name):
    """bn_stats aggregation -> sums -> this batch's AllReduce."""
    arin = small.tile([128, 4], F32, tag=f"ar_in_{name}")
    for ot in range(2):
        agg = small.tile([128, 2], F32, tag=f"{name}_agg")
        nc.vector.bn_aggr(agg[:], strip[:, ot, :])
        nc.vector.tensor_scalar(arin[:, 2 * ot:2 * ot + 1], agg[:, 0:1],
                                NSAMP, None, ALU.mult)
        m2 = small.tile([128, 1], F32, tag=f"{name}_m2")
        nc.vector.tensor_tensor(m2[:], agg[:, 0:1], agg[:, 0:1], ALU.mult)
        sqs = small.tile([128, 1], F32, tag=f"{name}_sq")
        nc.vector.tensor_tensor(sqs[:], agg[:, 1:2], m2[:], ALU.add)
        nc.vector.tensor_scalar(arin[:, 2 * ot + 1:2 * ot + 2], sqs[:],
                                NSAMP, None, ALU.mult)
    din = drampool.tile([128, 4], F32, tag=f"din_{name}")
    dout = drampool.tile([128, 4], F32, tag=f"dout_{name}")
    nc.gpsimd.dma_start(din[:], arin[:])
    nc.gpsimd.collective_compute(
        "AllReduce", ALU.add, replica_groups=[list(range(N_CORES))],
        ins=[din.opt()], outs=[dout.opt()])
    return dout


def _stats_finish(nc, small, ar_outs, gb_sb, name):
    """Combine the two per-batch AllReduce results into scale/shift."""
    ags = []
    for i, dout in enumerate(ar_outs):
        ag = small.tile([128, 4], F32, tag=f"{name}_ag{i}")
        nc.gpsimd.dma_start(ag[:], dout[:])
        ags.append(ag)
    tot = small.tile([128, 4], F32, tag=f"{name}_tot")
    nc.vector.tensor_tensor(tot[:], ags[0][:], ags[1][:], ALU.add)

    s_sb = small.tile([128, 2], F32, tag=f"{name}_s")
    t_sb = small.tile([128, 2], F32, tag=f"{name}_t")
    for ot in range(2):
        mean = small.tile([128, 1], F32, tag=f"{name}_mean")
        nc.vector.tensor_scalar(mean[:], tot[:, 2 * ot:2 * ot + 1],
                                1.0 / NTOT, None, ALU.mult)
        ey2 = small.tile([128, 1], F32, tag=f"{name}_ey2")
        nc.vector.tensor_scalar(ey2[:], tot[:, 2 * ot + 1:2 * ot + 2],
                                1.0 / NTOT, None, ALU.mult)
        m2 = small.tile([128, 1], F32, tag=f"{name}_gm2")
        nc.vector.tensor_tensor(m2[:], mean[:], mean[:], ALU.mult)
        x = small.tile([128, 1], F32, tag=f"{name}_x")
        nc.vector.scalar_tensor_tensor(x[:], ey2[:], EPS_BN, m2[:],
                                       ALU.add, ALU.subtract)
        # sqrt + 2 Newton steps (ACT Sqrt alone can be inaccurate)
        sd = small.tile([128, 1], F32, tag=f"{name}_sd")
        nc.scalar.activation(sd[:], x[:], AF.Sqrt)
        for it in range(2):
            rc = small.tile([128, 1], F32, tag=f"{name}_rc{it}")
            nc.vector.reciprocal(rc[:], sd[:])
            q = small.tile([128, 1], F32, tag=f"{name}_q{it}")
            nc.vector.tensor_tensor(q[:], x[:], rc[:], ALU.mult)
            u = small.tile([128, 1], F32, tag=f"{name}_u{it}")
            nc.vector.tensor_tensor(u[:], sd[:], q[:], ALU.add)
            sd = small.tile([128, 1], F32, tag=f"{name}_sd{it}")
            nc.vector.tensor_scalar(sd[:], u[:], 0.5, None, ALU.mult)
        inv = small.tile([128, 1], F32, tag=f"{name}_inv")
        nc.vector.reciprocal(inv[:], sd[:])
        nc.vector.tensor_tensor(s_sb[:, ot:ot + 1], inv[:],
                                gb_sb[:, 2 * ot:2 * ot + 1], ALU.mult)
        ms = small.tile([128, 1], F32, tag=f"{name}_ms")
        nc.vector.tensor_tensor(ms[:], mean[:], s_sb[:, ot:ot + 1], ALU.mult)
        nc.vector.tensor_tensor(t_sb[:, ot:ot + 1],
                                gb_sb[:, 2 * ot + 1:2 * ot + 2], ms[:],
                                ALU.subtract)
    return s_sb, t_sb


def _prep_core(xyz1, xyz2, points1, points2):
    """Host-side prep of one core's 2 batches."""
    B = xyz1.shape[0]
    x1s = np.zeros((B, KROWS, N), np.float16)
    x2s = np.zeros((B, KROWS, M), np.float16)
    for b in range(B):
        s1, s2 = _build_sides(xyz1[b], xyz2[b])
        x1s[b], x2s[b] = s1, s2
    p1T = np.ascontiguousarray(points1.transpose(0, 2, 1)).astype(np.float16)
    p2T = np.ascontiguousarray(points2.transpose(0, 2, 1)).astype(np.float16)
    return x1s, x2s, p1T, p2T


def _csr(p2, zw):
    """colsum(Z) over the ACT-Sign m-tiles only (m >= 128*N_DVE_MTS)."""
    m0 = 128 * N_DVE_MTS
    out = np.zeros((p2.shape[0], O), np.float16)
    for b in range(p2.shape[0]):
        cs = p2[b, m0:].astype(np.float16).astype(np.float32).sum(0) @ \
            zw.astype(np.float32)
        out[b] = cs.astype(np.float16)
    return out


def kernel(xyz1, xyz2, points1, points2, W1, b1, g1, beta1, W2, b2, g2,
           beta2):
    xyz1, xyz2 = np.asarray(xyz1), np.asarray(xyz2)
    points1, points2 = np.asarray(points1), np.asarray(points2)
    W1, W2 = np.asarray(W1, np.float32), np.asarray(W2, np.float32)
    g1, beta1 = np.asarray(g1, np.float32), np.asarray(beta1, np.float32)
    g2, beta2 = np.asarray(g2, np.float32), np.asarray(beta2, np.float32)
    # interpolation weight exactly as the reference computes it
    dist = np.float32(1e-10)
    inv = np.float32(1.0) / dist
    ssum = (inv + inv) + inv
    w = inv / ssum  # fp32(1/3)-ish, bit-exact vs reference

    zw = (0.5 * w * W1[:, :C2].astype(np.float32)).T.astype(np.float16)
    w1bT = np.ascontiguousarray(W1[:, C2:].T).astype(np.float16)
    w2T = np.ascontiguousarray(W2.T).astype(np.float16)
    # conv biases b1/b2 are no-ops through BN (mean subtracts them exactly)
    gb1 = np.stack([g1[0:128], beta1[0:128], g1[128:256], beta1[128:256]],
                   1).astype(np.float32)
    gb2 = np.stack([g2[0:128], beta2[0:128], g2[128:256], beta2[128:256]],
                   1).astype(np.float32)
    ident = np.eye(128, dtype=np.float16)

    nc = build_program()
    in_maps = []
    for c in range(N_CORES):
        bs = slice(c * B_PER_CORE, (c + 1) * B_PER_CORE)
        x1s, x2s, p1T, p2T = _prep_core(
            np.asarray(xyz1[bs]), np.asarray(xyz2[bs]),
            np.asarray(points1[bs]), np.asarray(points2[bs]))
        csr = _csr(np.asarray(points2[bs]), zw)
        in_maps.append(dict(x1s=x1s, x2s=x2s, p1T=p1T, p2T=p2T, zw=zw,
                            w1bT=w1bT, w2T=w2T, gb1=gb1, gb2=gb2, csr=csr,
                            ident=ident))
    res = bass_utils.run_bass_kernel_spmd(
        nc, in_maps, core_ids=list(range(N_CORES)), trace=False)
    out = np.concatenate([res.results[c]["out"] for c in range(N_CORES)],
                         axis=0)
    return out.astype(np.float32)
